# revision 7
# baseline (speedup 1.0000x reference)
"""Trainium2 Bass kernel for hierarchical-classifier (BHCN) forward + AWX pooling.

Math (per reference):
  l1  = x @ W0.T                            -> log_softmax -> lo[:, :32]
  a1  = LN(relu(l1));  l2m = [a1, x] @ W1.T -> log_softmax -> lo[:, 32:544]
  a2  = LN(relu(l2m)); l2  = [a2, x] @ W2.T -> log_softmax -> lo[:, 544:8736]
  s   = sigmoid(l2); pooled = (s*s) @ R.T
  awx = sqrt(clip(pooled, eps, 1-eps))

Sharding across 8 cores: 2 batch groups x 4 leaf shards. Each core runs the
small L1/L2 MLP for its 512-row batch group, then computes only ITS quarter of
the l2 columns (leaf shard j covers leaves [2048j, 2048j+2048)) in fp8
DoubleRow, and the partial AWX pooling s2_loc @ R[:, leaves_loc].T over ALL
classes (also fp8 DoubleRow). Each l2 column is computed exactly once
chip-wide. The host sums the 4 partial pooled tensors per batch group and
applies clip+sqrt, and normalizes the raw l2 block with a logsumexp computed
host-side from the returned bf16 l2 values. Device work is arranged to keep
the PE stream dense: transposes are batched per level / software-pipelined
behind the W2 matmuls, and psum drains are spread across Vector/Scalar/GpSimd.
"""

from contextlib import ExitStack

import numpy as np

_NC_CACHE: dict = {}

# Problem constants (hardcoded per contract; kernel.py must be self-contained).
B = 1024
D = 768
L0 = 32
L1 = 512
L2 = 8192
TOTAL = L0 + L1 + L2  # 8736
LN_EPS = 1e-5
AWX_EPS = 1e-6

N_CORES = 8
R_C = 4                      # leaf shards per batch group
R_B = N_CORES // R_C         # batch groups (2)
B_CORE = B // R_B            # rows per core (512)
B_TILES = B_CORE // 128      # 128-row tiles per core (4)
LEAF_LOC = L2 // R_C         # leaf columns per core (2048)
KT_LOC = LEAF_LOC // 128     # k-tiles of local s2T (16)
N_CH2 = LEAF_LOC // 512      # 512-wide W2 output chunks per core (4)
T_CHUNK = 512
N_TCH = (TOTAL + T_CHUNK - 1) // T_CHUNK   # pooled output chunks (18, tail 32)
W2_FP8 = True                # W2 matmul in fp8 DoubleRow (vs bf16)


def _build_nc():
    import concourse.bass as bass  # noqa: F401
    import concourse.tile as tile
    from concourse import bacc, mybir
    from concourse.masks import make_identity

    f32 = mybir.dt.float32
    bf16 = mybir.dt.bfloat16
    f8 = mybir.dt.float8e4
    AF = mybir.ActivationFunctionType
    ALU = mybir.AluOpType
    X = mybir.AxisListType.X
    DR = mybir.MatmulPerfMode.DoubleRow
    d_kt = D // 128           # 6 k-tiles in x
    l1_kt = L1 // 128         # 4 k-tiles in a2
    c_kt = d_kt + l1_kt       # 10 k-tiles for the W2 contraction

    nc = bacc.Bacc("TRN2", debug=False, target_bir_lowering=False)

    xTbf = nc.dram_tensor("xTbf", (D, B_CORE), bf16, kind="ExternalInput")
    w0T = nc.dram_tensor("w0T", (D, L0), bf16, kind="ExternalInput")
    w1T0 = nc.dram_tensor("w1T0", (L0, L1), bf16, kind="ExternalInput")
    w1T1 = nc.dram_tensor("w1T1", (D, L1), bf16, kind="ExternalInput")
    w2dt = f8 if W2_FP8 else bf16
    w2T = nc.dram_tensor("w2T", (L1 + D, LEAF_LOC), w2dt, kind="ExternalInput")
    if W2_FP8:
        xTf8 = nc.dram_tensor("xTf8", (D, B_CORE), f8, kind="ExternalInput")
        xTf8_r = xTf8.ap().rearrange("(ko p) b -> p ko b", p=128)
    rT = nc.dram_tensor("rT", (128, KT_LOC, TOTAL), f8, kind="ExternalInput")
    lo12 = nc.dram_tensor("lo12", (B_CORE, L0 + L1), bf16, kind="ExternalOutput")
    l2r = nc.dram_tensor("l2r", (B_CORE, LEAF_LOC), bf16, kind="ExternalOutput")
    pp = nc.dram_tensor("pp", (B_CORE, TOTAL), bf16, kind="ExternalOutput")

    xTbf_r = xTbf.ap().rearrange("(ko p) b -> p ko b", p=128)
    w0T_r = w0T.ap().rearrange("(ko p) n -> p ko n", p=128)
    w1T1_r = w1T1.ap().rearrange("(ko p) n -> p ko n", p=128)
    w2T_r = w2T.ap().rearrange("(ko p) n -> p ko n", p=128)

    with tile.TileContext(nc) as tc, ExitStack() as ctx:
        const = ctx.enter_context(tc.tile_pool(name="const", bufs=1))
        persist = ctx.enter_context(tc.tile_pool(name="persist", bufs=1))
        mlp = ctx.enter_context(tc.tile_pool(name="mlp", bufs=2))
        scratch = ctx.enter_context(tc.tile_pool(name="scratch", bufs=2))
        s2p = ctx.enter_context(tc.tile_pool(name="s2p", bufs=4))
        w2s = ctx.enter_context(tc.tile_pool(name="w2s", bufs=2))
        rts = ctx.enter_context(tc.tile_pool(name="rts", bufs=3))
        outp = ctx.enter_context(tc.tile_pool(name="outp", bufs=3))
        ps = ctx.enter_context(tc.tile_pool(name="ps", bufs=6, space="PSUM"))
        ps_tr = ctx.enter_context(tc.tile_pool(name="ps_tr", bufs=2, space="PSUM"))

        idbf = const.tile([128, 128], bf16, tag="idbf")
        make_identity(nc, idbf)
        eps_t = const.tile([128, 1], f32, tag="eps")
        nc.vector.memset(eps_t, LN_EPS)

        # Resident weights/activations
        xTbf_sb = const.tile([128, d_kt, B_CORE], bf16, tag="xTbf")
        nc.sync.dma_start(xTbf_sb[:], xTbf_r)
        w0T_sb = const.tile([128, d_kt, L0], bf16, tag="w0T")
        nc.sync.dma_start(w0T_sb[:], w0T_r)
        w1T0_sb = const.tile([L0, L1], bf16, tag="w1T0")
        nc.sync.dma_start(w1T0_sb[:], w1T0.ap())
        w1T1_sb = const.tile([128, d_kt, L1], bf16, tag="w1T1")
        nc.sync.dma_start(w1T1_sb[:], w1T1_r)
        if W2_FP8:
            xTf8_sb = const.tile([128, d_kt, B_CORE], f8, tag="xTf8")
            nc.sync.dma_start(xTf8_sb[:], xTf8_r)

        s2T_sb = [persist.tile([128, KT_LOC, 128], f8, tag=f"s2T{bt}",
                               name=f"s2T{bt}")
                  for bt in range(B_TILES)]
        if W2_FP8:
            a2xT = [persist.tile([128, l1_kt, 128], f8, tag=f"a2xT{bt}",
                                 name=f"a2xT{bt}")
                    for bt in range(B_TILES)]
        else:
            hn2Ts = [persist.tile([128, l1_kt, 128], bf16, tag=f"hn2T{bt}",
                                  name=f"hn2T{bt}")
                     for bt in range(B_TILES)]

        def log_softmax_small(ps_t, width, rsl, col0):
            """log_softmax over `width` free elems from PSUM; DMA bf16 to lo12."""
            mneg = mlp.tile([128, 1], f32, tag="mneg")
            nc.vector.tensor_reduce(mneg, ps_t, axis=X, op=ALU.max, negate=True)
            e_t = scratch.tile([128, 512], f32, tag="sgs", name="e_t")[:, :width]
            ssum = mlp.tile([128, 1], f32, tag="ssum")
            nc.scalar.activation(e_t, ps_t, AF.Exp, bias=mneg, accum_out=ssum)
            lse = mlp.tile([128, 1], f32, tag="lse")
            nc.scalar.activation(lse, ssum, AF.Ln)
            csub = mlp.tile([128, 1], f32, tag="csub")
            nc.vector.tensor_sub(csub, lse, mneg)  # lse + max
            lov = scratch.tile([128, 512], bf16, tag="lov", name="lov")[:, :width]
            nc.vector.tensor_scalar_sub(lov, ps_t, csub)
            nc.scalar.dma_start(lo12.ap()[rsl, col0:col0 + width], lov)

        def layer_norm_relu(ps_t, width):
            """returns hn = LN(relu(ps)) tile [128, width] (fp32)."""
            h = mlp.tile([128, 512], f32, tag="h", name="h")[:, :width]
            nc.vector.tensor_scalar_max(h, ps_t, 0.0)
            stats = mlp.tile([128, 6], f32, tag="stats")
            nc.vector.bn_stats(stats, h)
            mv = mlp.tile([128, 2], f32, tag="mv")
            nc.vector.bn_aggr(mv, stats)
            lnv = mlp.tile([128, 1], f32, tag="lnv")
            nc.scalar.activation(lnv, mv[:, 1:2], AF.Ln, bias=eps_t)
            rstd = mlp.tile([128, 1], f32, tag="rstd")
            nc.scalar.activation(rstd, lnv, AF.Exp, scale=-0.5)
            nc.vector.tensor_scalar(h, h, mv[:, 0:1], rstd,
                                    op0=ALU.subtract, op1=ALU.mult)
            return h

        # ---- Level 1: all batch tiles, then transposes batched ----
        hn1bfs = []
        for bt in range(B_TILES):
            bsl = slice(bt * 128, (bt + 1) * 128)
            ps_a = ps.tile([128, 512], f32, tag="ps", name="ps_a")[:, :L0]
            for ko in range(d_kt):
                nc.tensor.matmul(ps_a, xTbf_sb[:, ko, bsl], w0T_sb[:, ko, :],
                                 start=(ko == 0), stop=(ko == d_kt - 1))
            log_softmax_small(ps_a, L0, bsl, 0)
            hn1 = layer_norm_relu(ps_a, L0)
            hn1bf = mlp.tile([128, L0], bf16, tag="hn1bf", name=f"hn1bf{bt}")
            nc.vector.tensor_copy(hn1bf, hn1)
            hn1bfs.append(hn1bf)
        hn1Ts = []
        for bt in range(B_TILES):
            pt = ps_tr.tile([128, 128], bf16, tag="pt", name="pt_a")[:L0, :]
            nc.tensor.transpose(pt, hn1bfs[bt], idbf)
            hn1T = mlp.tile([L0, 128], bf16, tag="hn1T", name=f"hn1T{bt}")
            nc.vector.tensor_copy(hn1T, pt)
            hn1Ts.append(hn1T)

        # ---- Level 2: all batch tiles, then transposes batched ----
        hn2bfs = []
        for bt in range(B_TILES):
            bsl = slice(bt * 128, (bt + 1) * 128)
            ps_b = ps.tile([128, 512], f32, tag="ps", name="ps_b")
            nc.tensor.matmul(ps_b, hn1Ts[bt], w1T0_sb[:], start=True, stop=False)
            for ko in range(d_kt):
                nc.tensor.matmul(ps_b, xTbf_sb[:, ko, bsl], w1T1_sb[:, ko, :],
                                 start=False, stop=(ko == d_kt - 1))
            log_softmax_small(ps_b, L1, bsl, L0)
            hn2 = layer_norm_relu(ps_b, L1)
            hn2bf = mlp.tile([128, L1], bf16, tag="hn2bf", name=f"hn2bf{bt}")
            nc.vector.tensor_copy(hn2bf, hn2)
            hn2bfs.append(hn2bf)
        for bt in range(B_TILES):
            for j in range(l1_kt):
                pt = ps_tr.tile([128, 128], bf16, tag="pt", name="pt_b")
                nc.tensor.transpose(pt, hn2bfs[bt][:, j * 128:(j + 1) * 128],
                                    idbf)
                dst = a2xT[bt][:, j, :] if W2_FP8 else hn2Ts[bt][:, j, :]
                if j % 2 == 0:
                    nc.vector.tensor_copy(dst, pt)
                else:
                    nc.scalar.copy(dst, pt)

        # ---- Level 3: l2 chunk = [a2, x] @ W2T[:, chunk] for local leaves ----
        # Transposes of s2 are software-pipelined 2 slots behind the matmuls so
        # the PE never waits on the sigmoid drain chain.
        tr_pending = []

        def flush_tr():
            s2bf_t, bt_, nci_ = tr_pending.pop(0)
            for j in range(0, 4, 2):
                pt2 = ps_tr.tile([128, 2, 128], bf16, tag="pt", name="pt_s2")
                nc.tensor.transpose(pt2[:, 0, :],
                                    s2bf_t[:, j * 128:(j + 1) * 128], idbf)
                nc.tensor.transpose(pt2[:, 1, :],
                                    s2bf_t[:, (j + 1) * 128:(j + 2) * 128],
                                    idbf)
                dst = s2T_sb[bt_][:, nci_ * 4 + j:nci_ * 4 + j + 2, :]
                if j == 0:
                    nc.vector.tensor_copy(dst, pt2)
                else:
                    nc.scalar.copy(dst, pt2)

        for nci in range(N_CH2):
            nsl = slice(nci * 512, (nci + 1) * 512)
            w2t_t = w2s.tile([128, c_kt, 512], w2dt, tag="w2t")
            nc.sync.dma_start(w2t_t[:], w2T_r[:, :, nsl])
            for bt in range(B_TILES):
                bsl = slice(bt * 128, (bt + 1) * 128)
                ps_c = ps.tile([128, 512], f32, tag="ps", name="ps_c")
                if W2_FP8:
                    for ko in range(0, l1_kt, 2):
                        nc.tensor.matmul(ps_c, a2xT[bt][:, ko:ko + 2, :],
                                         w2t_t[:, ko:ko + 2, :],
                                         start=(ko == 0), stop=False,
                                         perf_mode=DR)
                    for ko in range(0, d_kt, 2):
                        k0 = l1_kt + ko
                        nc.tensor.matmul(ps_c, xTf8_sb[:, ko:ko + 2, bsl],
                                         w2t_t[:, k0:k0 + 2, :],
                                         start=False, stop=(ko == d_kt - 2),
                                         perf_mode=DR)
                else:
                    for ko in range(c_kt):
                        lhsT = (hn2Ts[bt][:, ko, :] if ko < l1_kt
                                else xTbf_sb[:, ko - l1_kt, bsl])
                        nc.tensor.matmul(ps_c, lhsT, w2t_t[:, ko, :],
                                         start=(ko == 0), stop=(ko == c_kt - 1))
                # raw l2 out in bf16 (host computes lse + applies it)
                l2bf = outp.tile([128, 512], bf16, tag="l2bf", name="l2bf")
                nc.scalar.copy(l2bf, ps_c)
                nc.scalar.dma_start(l2r.ap()[bsl, nsl], l2bf)
                # s^2 = sigmoid(l2)^2 straight from PSUM
                sg = scratch.tile([128, 512], f32, tag="sgs", name="sg")
                nc.scalar.activation(sg, ps_c, AF.Exp, scale=-1.0)
                nc.gpsimd.tensor_scalar_add(sg, sg, 1.0)
                nc.vector.reciprocal_approx_fast(sg, sg)
                s2bf = s2p.tile([128, 512], bf16, tag="s2bf", name="s2bf")
                nc.gpsimd.tensor_mul(s2bf, sg, sg)
                tr_pending.append((s2bf, bt, nci))
                if len(tr_pending) > 2:
                    flush_tr()
        while tr_pending:
            flush_tr()

        # ---- partial AWX: pp = s2_loc @ R_loc.T over all classes ----
        for tci in range(N_TCH):
            t0c = tci * T_CHUNK
            tw = min(T_CHUNK, TOTAL - t0c)
            rt_full = rts.tile([128, KT_LOC, T_CHUNK], f8, tag="rt", name="rt")
            rt_t = rt_full[:, :, :tw]
            nc.sync.dma_start(rt_t, rT.ap()[:, :, t0c:t0c + tw])
            for bt in range(B_TILES):
                bsl = slice(bt * 128, (bt + 1) * 128)
                ps_p = ps.tile([128, T_CHUNK], f32, tag="ps",
                               name=f"pp{tci}_{bt}")[:, :tw]
                for ko in range(0, KT_LOC, 2):
                    nc.tensor.matmul(ps_p, s2T_sb[bt][:, ko:ko + 2, :],
                                     rt_t[:, ko:ko + 2, :],
                                     start=(ko == 0), stop=(ko == KT_LOC - 2),
                                     perf_mode=DR)
                ob = outp.tile([128, T_CHUNK], bf16, tag="ob",
                               name="ob")[:, :tw]
                if bt % 2 == 0:
                    nc.vector.tensor_copy(ob, ps_p)
                else:
                    nc.scalar.copy(ob, ps_p)
                nc.scalar.dma_start(pp.ap()[bsl, t0c:t0c + tw], ob)

    nc.compile()
    return nc


def _get_nc():
    if "nc" not in _NC_CACHE:
        _NC_CACHE["nc"] = _build_nc()
    return _NC_CACHE["nc"]


def _tile_rt(rt_loc):
    """(LEAF_LOC, TOTAL) 0/1 -> (128, KT_LOC, TOTAL) fp8, k = ko*128 + p."""
    import ml_dtypes
    v = rt_loc.reshape(KT_LOC, 128, TOTAL)
    return np.ascontiguousarray(v.transpose(1, 0, 2)).astype(
        ml_dtypes.float8_e4m3)


def _prep_in_maps(x, W0, W1, W2, R):
    import ml_dtypes
    bf = ml_dtypes.bfloat16
    f8 = ml_dtypes.float8_e4m3

    xT = np.ascontiguousarray(x.T, dtype=np.float32)          # (768, 1024)
    W0T = np.ascontiguousarray(W0.T).astype(bf)               # (768, 32)
    W1T = np.ascontiguousarray(W1.T, dtype=np.float32)        # (800, 512)
    W1T0 = np.ascontiguousarray(W1T[:L0]).astype(bf)
    W1T1 = np.ascontiguousarray(W1T[L0:]).astype(bf)
    # device concat order is [a2, x] -> W2T rows are [hn part; x part] already
    w2dt = f8 if W2_FP8 else bf
    W2T = np.ascontiguousarray(W2.T).astype(w2dt)             # (1280, 8192)
    RT = np.ascontiguousarray(R.T, dtype=np.float32)          # (8192, 8736)

    rt_shards = [_tile_rt(np.ascontiguousarray(
        RT[j * LEAF_LOC:(j + 1) * LEAF_LOC])) for j in range(R_C)]
    w2_shards = [np.ascontiguousarray(W2T[:, j * LEAF_LOC:(j + 1) * LEAF_LOC])
                 for j in range(R_C)]

    in_maps = []
    for c in range(N_CORES):
        g, j = divmod(c, R_C)
        cols = slice(g * B_CORE, (g + 1) * B_CORE)
        xTs = np.ascontiguousarray(xT[:, cols])
        m = {
            "xTbf": xTs.astype(bf),
            "w0T": W0T,
            "w1T0": W1T0,
            "w1T1": W1T1,
            "w2T": w2_shards[j],
            "rT": rt_shards[j],
        }
        if W2_FP8:
            m["xTf8"] = xTs.astype(f8)
        in_maps.append(m)
    return in_maps


def _run(x, W0, b0, W1, b1, W2, b2, R, trace=False):
    from concourse.bass_utils import run_bass_kernel_spmd

    for b_arr in (b0, b1, b2):
        assert np.abs(np.asarray(b_arr)).max() == 0.0, \
            "kernel assumes zero biases (as produced by setup_inputs)"

    in_maps = _prep_in_maps(np.asarray(x, np.float32), np.asarray(W0),
                            np.asarray(W1), np.asarray(W2), np.asarray(R))
    nc = _get_nc()
    res = run_bass_kernel_spmd(nc, in_maps, list(range(N_CORES)), trace=trace)

    lo_full = np.empty((B, TOTAL), np.float32)
    awx_full = np.empty((B, TOTAL), np.float32)
    for g in range(R_B):
        rows = slice(g * B_CORE, (g + 1) * B_CORE)
        cores = [g * R_C + j for j in range(R_C)]
        lo_full[rows, :L0 + L1] = np.asarray(
            res.results[cores[0]]["lo12"], np.float32)
        l2 = np.concatenate(
            [np.asarray(res.results[c]["l2r"], np.float32) for c in cores],
            axis=1)  # (B_CORE, 8192)
        m = l2.max(axis=1, keepdims=True)
        lse = m + np.log(np.exp(l2 - m).sum(axis=1, keepdims=True))
        lo_full[rows, L0 + L1:] = l2 - lse
        pooled = np.asarray(res.results[cores[0]]["pp"], np.float32)
        for c in cores[1:]:
            pooled += np.asarray(res.results[c]["pp"], np.float32)
        awx_full[rows] = np.sqrt(np.clip(pooled, AWX_EPS, 1.0 - AWX_EPS))
    return (lo_full, awx_full), res


def kernel(x, W0, b0, W1, b1, W2, b2, R):
    out, _ = _run(x, W0, b0, W1, b1, W2, b2, R, trace=False)
    return out


# revision 8
# speedup vs baseline: 1.4209x; 1.4209x over previous
"""Trainium2 Bass kernel for hierarchical-classifier (BHCN) forward + AWX pooling.

Math (per reference):
  l1  = x @ W0.T                            -> log_softmax -> lo[:, :32]
  a1  = LN(relu(l1));  l2m = [a1, x] @ W1.T -> log_softmax -> lo[:, 32:544]
  a2  = LN(relu(l2m)); l2  = [a2, x] @ W2.T -> log_softmax -> lo[:, 544:8736]
  s   = sigmoid(l2); pooled = (s*s) @ R.T
  awx = sqrt(clip(pooled, eps, 1-eps))

Sharding across 8 cores: 2 batch groups x 4 leaf shards. Each core runs the
small L1/L2 MLP for its 512-row batch group, then computes only ITS quarter of
the l2 columns (leaf shard j covers leaves [2048j, 2048j+2048)) in fp8
DoubleRow, and the partial AWX pooling s2_loc @ R[:, leaves_loc].T over ALL
classes (also fp8 DoubleRow). Each l2 column is computed exactly once
chip-wide. The host sums the 4 partial pooled tensors per batch group and
applies clip+sqrt, and normalizes the raw l2 block with a logsumexp computed
host-side from the returned bf16 l2 values. Device work is arranged to keep
the PE stream dense: transposes are batched per level / software-pipelined
behind the W2 matmuls, and psum drains are spread across Vector/Scalar/GpSimd.
"""

from contextlib import ExitStack

import numpy as np

_NC_CACHE: dict = {}

# Problem constants (hardcoded per contract; kernel.py must be self-contained).
B = 1024
D = 768
L0 = 32
L1 = 512
L2 = 8192
TOTAL = L0 + L1 + L2  # 8736
LN_EPS = 1e-5
AWX_EPS = 1e-6

N_CORES = 8
R_C = 4                      # leaf shards per batch group
R_B = N_CORES // R_C         # batch groups (2)
B_CORE = B // R_B            # rows per core (512)
B_TILES = B_CORE // 128      # 128-row tiles per core (4)
LEAF_LOC = L2 // R_C         # leaf columns per core (2048)
KT_LOC = LEAF_LOC // 128     # k-tiles of local s2T (16)
N_CH2 = LEAF_LOC // 512      # 512-wide W2 output chunks per core (4)
T_CHUNK = 512
N_TCH = (TOTAL + T_CHUNK - 1) // T_CHUNK   # pooled output chunks (18, tail 32)
W2_FP8 = True                # W2 matmul in fp8 DoubleRow (vs bf16)


def _build_nc():
    import concourse.bass as bass  # noqa: F401
    import concourse.tile as tile
    from concourse import bacc, mybir
    from concourse.masks import make_identity

    f32 = mybir.dt.float32
    bf16 = mybir.dt.bfloat16
    f8 = mybir.dt.float8e4
    AF = mybir.ActivationFunctionType
    ALU = mybir.AluOpType
    X = mybir.AxisListType.X
    DR = mybir.MatmulPerfMode.DoubleRow
    d_kt = D // 128           # 6 k-tiles in x
    l1_kt = L1 // 128         # 4 k-tiles in a2
    c_kt = d_kt + l1_kt       # 10 k-tiles for the W2 contraction

    nc = bacc.Bacc("TRN2", debug=False, target_bir_lowering=False)

    xTbf = nc.dram_tensor("xTbf", (D, B_CORE), bf16, kind="ExternalInput")
    w0T = nc.dram_tensor("w0T", (D, L0), bf16, kind="ExternalInput")
    w1T0 = nc.dram_tensor("w1T0", (L0, L1), bf16, kind="ExternalInput")
    w1T1 = nc.dram_tensor("w1T1", (D, L1), bf16, kind="ExternalInput")
    w2dt = f8 if W2_FP8 else bf16
    w2T = nc.dram_tensor("w2T", (L1 + D, LEAF_LOC), w2dt, kind="ExternalInput")
    if W2_FP8:
        xTf8 = nc.dram_tensor("xTf8", (D, B_CORE), f8, kind="ExternalInput")
        xTf8_r = xTf8.ap().rearrange("(ko p) b -> p ko b", p=128)
    rT = nc.dram_tensor("rT", (128, KT_LOC, TOTAL), f8, kind="ExternalInput")
    lo12 = nc.dram_tensor("lo12", (B_CORE, L0 + L1), bf16, kind="ExternalOutput")
    l2r = nc.dram_tensor("l2r", (B_CORE, LEAF_LOC), bf16, kind="ExternalOutput")
    pp = nc.dram_tensor("pp", (B_CORE, TOTAL), bf16, kind="ExternalOutput")

    xTbf_r = xTbf.ap().rearrange("(ko p) b -> p ko b", p=128)
    w0T_r = w0T.ap().rearrange("(ko p) n -> p ko n", p=128)
    w1T1_r = w1T1.ap().rearrange("(ko p) n -> p ko n", p=128)
    w2T_r = w2T.ap().rearrange("(ko p) n -> p ko n", p=128)

    with tile.TileContext(nc) as tc, ExitStack() as ctx:
        const = ctx.enter_context(tc.tile_pool(name="const", bufs=1))
        persist = ctx.enter_context(tc.tile_pool(name="persist", bufs=1))
        mlp = ctx.enter_context(tc.tile_pool(name="mlp", bufs=2))
        scratch = ctx.enter_context(tc.tile_pool(name="scratch", bufs=2))
        s2p = ctx.enter_context(tc.tile_pool(name="s2p", bufs=4))
        w2s = ctx.enter_context(tc.tile_pool(name="w2s", bufs=2))
        rts = ctx.enter_context(tc.tile_pool(name="rts", bufs=3))
        outp = ctx.enter_context(tc.tile_pool(name="outp", bufs=3))
        ps = ctx.enter_context(tc.tile_pool(name="ps", bufs=6, space="PSUM"))
        ps_tr = ctx.enter_context(tc.tile_pool(name="ps_tr", bufs=2, space="PSUM"))

        idbf = const.tile([128, 128], bf16, tag="idbf")
        make_identity(nc, idbf)
        eps_t = const.tile([128, 1], f32, tag="eps")
        nc.vector.memset(eps_t, LN_EPS)

        # Resident weights/activations
        xTbf_sb = const.tile([128, d_kt, B_CORE], bf16, tag="xTbf")
        nc.sync.dma_start(xTbf_sb[:], xTbf_r)
        w0T_sb = const.tile([128, d_kt, L0], bf16, tag="w0T")
        nc.sync.dma_start(w0T_sb[:], w0T_r)
        w1T0_sb = const.tile([L0, L1], bf16, tag="w1T0")
        nc.sync.dma_start(w1T0_sb[:], w1T0.ap())
        w1T1_sb = const.tile([128, d_kt, L1], bf16, tag="w1T1")
        nc.sync.dma_start(w1T1_sb[:], w1T1_r)
        if W2_FP8:
            xTf8_sb = const.tile([128, d_kt, B_CORE], f8, tag="xTf8")
            nc.sync.dma_start(xTf8_sb[:], xTf8_r)

        s2T_sb = [persist.tile([128, KT_LOC, 128], f8, tag=f"s2T{bt}",
                               name=f"s2T{bt}")
                  for bt in range(B_TILES)]
        if W2_FP8:
            a2xT = [persist.tile([128, l1_kt, 128], f8, tag=f"a2xT{bt}",
                                 name=f"a2xT{bt}")
                    for bt in range(B_TILES)]
        else:
            hn2Ts = [persist.tile([128, l1_kt, 128], bf16, tag=f"hn2T{bt}",
                                  name=f"hn2T{bt}")
                     for bt in range(B_TILES)]

        def log_softmax_small(ps_t, width, rsl, col0):
            """log_softmax over `width` free elems from PSUM; DMA bf16 to lo12."""
            mneg = mlp.tile([128, 1], f32, tag="mneg")
            nc.vector.tensor_reduce(mneg, ps_t, axis=X, op=ALU.max, negate=True)
            e_t = scratch.tile([128, 512], f32, tag="sgs", name="e_t")[:, :width]
            ssum = mlp.tile([128, 1], f32, tag="ssum")
            nc.scalar.activation(e_t, ps_t, AF.Exp, bias=mneg, accum_out=ssum)
            lse = mlp.tile([128, 1], f32, tag="lse")
            nc.scalar.activation(lse, ssum, AF.Ln)
            csub = mlp.tile([128, 1], f32, tag="csub")
            nc.vector.tensor_sub(csub, lse, mneg)  # lse + max
            lov = scratch.tile([128, 512], bf16, tag="lov", name="lov")[:, :width]
            nc.vector.tensor_scalar_sub(lov, ps_t, csub)
            nc.scalar.dma_start(lo12.ap()[rsl, col0:col0 + width], lov)

        def layer_norm_relu(ps_t, width):
            """returns hn = LN(relu(ps)) tile [128, width] (fp32)."""
            h = mlp.tile([128, 512], f32, tag="h", name="h")[:, :width]
            nc.vector.tensor_scalar_max(h, ps_t, 0.0)
            stats = mlp.tile([128, 6], f32, tag="stats")
            nc.vector.bn_stats(stats, h)
            mv = mlp.tile([128, 2], f32, tag="mv")
            nc.vector.bn_aggr(mv, stats)
            lnv = mlp.tile([128, 1], f32, tag="lnv")
            nc.scalar.activation(lnv, mv[:, 1:2], AF.Ln, bias=eps_t)
            rstd = mlp.tile([128, 1], f32, tag="rstd")
            nc.scalar.activation(rstd, lnv, AF.Exp, scale=-0.5)
            nc.vector.tensor_scalar(h, h, mv[:, 0:1], rstd,
                                    op0=ALU.subtract, op1=ALU.mult)
            return h

        # ---- Level 1: all batch tiles, then transposes batched ----
        hn1bfs = []
        for bt in range(B_TILES):
            bsl = slice(bt * 128, (bt + 1) * 128)
            ps_a = ps.tile([128, 512], f32, tag="ps", name="ps_a")[:, :L0]
            for ko in range(d_kt):
                nc.tensor.matmul(ps_a, xTbf_sb[:, ko, bsl], w0T_sb[:, ko, :],
                                 start=(ko == 0), stop=(ko == d_kt - 1))
            log_softmax_small(ps_a, L0, bsl, 0)
            hn1 = layer_norm_relu(ps_a, L0)
            hn1bf = mlp.tile([128, L0], bf16, tag="hn1bf", name=f"hn1bf{bt}")
            nc.vector.tensor_copy(hn1bf, hn1)
            hn1bfs.append(hn1bf)
        hn1Ts = []
        for bt in range(B_TILES):
            pt = ps_tr.tile([128, 128], bf16, tag="pt", name="pt_a")[:L0, :]
            nc.tensor.transpose(pt, hn1bfs[bt], idbf)
            hn1T = mlp.tile([L0, 128], bf16, tag="hn1T", name=f"hn1T{bt}")
            nc.vector.tensor_copy(hn1T, pt)
            hn1Ts.append(hn1T)

        # ---- Level 2: all batch tiles, then transposes batched ----
        hn2bfs = []
        for bt in range(B_TILES):
            bsl = slice(bt * 128, (bt + 1) * 128)
            ps_b = ps.tile([128, 512], f32, tag="ps", name="ps_b")
            nc.tensor.matmul(ps_b, hn1Ts[bt], w1T0_sb[:], start=True, stop=False)
            for ko in range(d_kt):
                nc.tensor.matmul(ps_b, xTbf_sb[:, ko, bsl], w1T1_sb[:, ko, :],
                                 start=False, stop=(ko == d_kt - 1))
            log_softmax_small(ps_b, L1, bsl, L0)
            hn2 = layer_norm_relu(ps_b, L1)
            hn2bf = mlp.tile([128, L1], bf16, tag="hn2bf", name=f"hn2bf{bt}")
            nc.vector.tensor_copy(hn2bf, hn2)
            hn2bfs.append(hn2bf)
        for bt in range(B_TILES):
            for j in range(l1_kt):
                pt = ps_tr.tile([128, 128], bf16, tag="pt", name="pt_b")
                nc.tensor.transpose(pt, hn2bfs[bt][:, j * 128:(j + 1) * 128],
                                    idbf)
                dst = a2xT[bt][:, j, :] if W2_FP8 else hn2Ts[bt][:, j, :]
                if j % 2 == 0:
                    nc.vector.tensor_copy(dst, pt)
                else:
                    nc.scalar.copy(dst, pt)

        # ---- Level 3: l2 chunk = [a2, x] @ W2T[:, chunk] for local leaves ----
        # Transposes of s2 are software-pipelined 2 slots behind the matmuls so
        # the PE never waits on the sigmoid drain chain.
        tr_pending = []

        def flush_tr():
            s2bf_t, bt_, nci_ = tr_pending.pop(0)
            for j in range(0, 4, 2):
                pt2 = ps_tr.tile([128, 2, 128], bf16, tag="pt", name="pt_s2")
                nc.tensor.transpose(pt2[:, 0, :],
                                    s2bf_t[:, j * 128:(j + 1) * 128], idbf)
                nc.tensor.transpose(pt2[:, 1, :],
                                    s2bf_t[:, (j + 1) * 128:(j + 2) * 128],
                                    idbf)
                dst = s2T_sb[bt_][:, nci_ * 4 + j:nci_ * 4 + j + 2, :]
                if j == 0:
                    nc.vector.tensor_copy(dst, pt2)
                else:
                    nc.scalar.copy(dst, pt2)

        for nci in range(N_CH2):
            nsl = slice(nci * 512, (nci + 1) * 512)
            w2t_t = w2s.tile([128, c_kt, 512], w2dt, tag="w2t")
            nc.sync.dma_start(w2t_t[:], w2T_r[:, :, nsl])
            for bt in range(B_TILES):
                bsl = slice(bt * 128, (bt + 1) * 128)
                ps_c = ps.tile([128, 512], f32, tag="ps", name="ps_c")
                if W2_FP8:
                    for ko in range(0, l1_kt, 2):
                        nc.tensor.matmul(ps_c, a2xT[bt][:, ko:ko + 2, :],
                                         w2t_t[:, ko:ko + 2, :],
                                         start=(ko == 0), stop=False,
                                         perf_mode=DR)
                    for ko in range(0, d_kt, 2):
                        k0 = l1_kt + ko
                        nc.tensor.matmul(ps_c, xTf8_sb[:, ko:ko + 2, bsl],
                                         w2t_t[:, k0:k0 + 2, :],
                                         start=False, stop=(ko == d_kt - 2),
                                         perf_mode=DR)
                else:
                    for ko in range(c_kt):
                        lhsT = (hn2Ts[bt][:, ko, :] if ko < l1_kt
                                else xTbf_sb[:, ko - l1_kt, bsl])
                        nc.tensor.matmul(ps_c, lhsT, w2t_t[:, ko, :],
                                         start=(ko == 0), stop=(ko == c_kt - 1))
                # raw l2 out in bf16 (host computes lse + applies it)
                l2bf = outp.tile([128, 512], bf16, tag="l2bf", name="l2bf")
                nc.scalar.copy(l2bf, ps_c)
                nc.scalar.dma_start(l2r.ap()[bsl, nsl], l2bf)
                # s^2 = sigmoid(l2)^2 straight from PSUM
                sg = scratch.tile([128, 512], f32, tag="sgs", name="sg")
                nc.scalar.activation(sg, ps_c, AF.Exp, scale=-1.0)
                nc.vector.tensor_scalar_add(sg, sg, 1.0)
                nc.vector.reciprocal_approx_fast(sg, sg)
                s2bf = s2p.tile([128, 512], bf16, tag="s2bf", name="s2bf")
                nc.vector.tensor_mul(s2bf, sg, sg)
                tr_pending.append((s2bf, bt, nci))
                if len(tr_pending) > 2:
                    flush_tr()
        while tr_pending:
            flush_tr()

        # ---- partial AWX: pp = s2_loc @ R_loc.T over all classes ----
        for tci in range(N_TCH):
            t0c = tci * T_CHUNK
            tw = min(T_CHUNK, TOTAL - t0c)
            rt_full = rts.tile([128, KT_LOC, T_CHUNK], f8, tag="rt", name="rt")
            rt_t = rt_full[:, :, :tw]
            nc.sync.dma_start(rt_t, rT.ap()[:, :, t0c:t0c + tw])
            for bt in range(B_TILES):
                bsl = slice(bt * 128, (bt + 1) * 128)
                ps_p = ps.tile([128, T_CHUNK], f32, tag="ps",
                               name=f"pp{tci}_{bt}")[:, :tw]
                for ko in range(0, KT_LOC, 2):
                    nc.tensor.matmul(ps_p, s2T_sb[bt][:, ko:ko + 2, :],
                                     rt_t[:, ko:ko + 2, :],
                                     start=(ko == 0), stop=(ko == KT_LOC - 2),
                                     perf_mode=DR)
                ob = outp.tile([128, T_CHUNK], bf16, tag="ob",
                               name="ob")[:, :tw]
                if bt % 2 == 0:
                    nc.vector.tensor_copy(ob, ps_p)
                else:
                    nc.scalar.copy(ob, ps_p)
                nc.scalar.dma_start(pp.ap()[bsl, t0c:t0c + tw], ob)

    nc.compile()
    return nc


def _get_nc():
    if "nc" not in _NC_CACHE:
        _NC_CACHE["nc"] = _build_nc()
    return _NC_CACHE["nc"]


def _tile_rt(rt_loc):
    """(LEAF_LOC, TOTAL) 0/1 -> (128, KT_LOC, TOTAL) fp8, k = ko*128 + p."""
    import ml_dtypes
    v = rt_loc.reshape(KT_LOC, 128, TOTAL)
    return np.ascontiguousarray(v.transpose(1, 0, 2)).astype(
        ml_dtypes.float8_e4m3)


def _prep_in_maps(x, W0, W1, W2, R):
    import ml_dtypes
    bf = ml_dtypes.bfloat16
    f8 = ml_dtypes.float8_e4m3

    xT = np.ascontiguousarray(x.T, dtype=np.float32)          # (768, 1024)
    W0T = np.ascontiguousarray(W0.T).astype(bf)               # (768, 32)
    W1T = np.ascontiguousarray(W1.T, dtype=np.float32)        # (800, 512)
    W1T0 = np.ascontiguousarray(W1T[:L0]).astype(bf)
    W1T1 = np.ascontiguousarray(W1T[L0:]).astype(bf)
    # device concat order is [a2, x] -> W2T rows are [hn part; x part] already
    w2dt = f8 if W2_FP8 else bf
    W2T = np.ascontiguousarray(W2.T).astype(w2dt)             # (1280, 8192)
    RT = np.ascontiguousarray(R.T, dtype=np.float32)          # (8192, 8736)

    rt_shards = [_tile_rt(np.ascontiguousarray(
        RT[j * LEAF_LOC:(j + 1) * LEAF_LOC])) for j in range(R_C)]
    w2_shards = [np.ascontiguousarray(W2T[:, j * LEAF_LOC:(j + 1) * LEAF_LOC])
                 for j in range(R_C)]

    in_maps = []
    for c in range(N_CORES):
        g, j = divmod(c, R_C)
        cols = slice(g * B_CORE, (g + 1) * B_CORE)
        xTs = np.ascontiguousarray(xT[:, cols])
        m = {
            "xTbf": xTs.astype(bf),
            "w0T": W0T,
            "w1T0": W1T0,
            "w1T1": W1T1,
            "w2T": w2_shards[j],
            "rT": rt_shards[j],
        }
        if W2_FP8:
            m["xTf8"] = xTs.astype(f8)
        in_maps.append(m)
    return in_maps


def _run(x, W0, b0, W1, b1, W2, b2, R, trace=False):
    from concourse.bass_utils import run_bass_kernel_spmd

    for b_arr in (b0, b1, b2):
        assert np.abs(np.asarray(b_arr)).max() == 0.0, \
            "kernel assumes zero biases (as produced by setup_inputs)"

    in_maps = _prep_in_maps(np.asarray(x, np.float32), np.asarray(W0),
                            np.asarray(W1), np.asarray(W2), np.asarray(R))
    nc = _get_nc()
    res = run_bass_kernel_spmd(nc, in_maps, list(range(N_CORES)), trace=trace)

    lo_full = np.empty((B, TOTAL), np.float32)
    awx_full = np.empty((B, TOTAL), np.float32)
    for g in range(R_B):
        rows = slice(g * B_CORE, (g + 1) * B_CORE)
        cores = [g * R_C + j for j in range(R_C)]
        lo_full[rows, :L0 + L1] = np.asarray(
            res.results[cores[0]]["lo12"], np.float32)
        l2 = np.concatenate(
            [np.asarray(res.results[c]["l2r"], np.float32) for c in cores],
            axis=1)  # (B_CORE, 8192)
        m = l2.max(axis=1, keepdims=True)
        lse = m + np.log(np.exp(l2 - m).sum(axis=1, keepdims=True))
        lo_full[rows, L0 + L1:] = l2 - lse
        pooled = np.asarray(res.results[cores[0]]["pp"], np.float32)
        for c in cores[1:]:
            pooled += np.asarray(res.results[c]["pp"], np.float32)
        awx_full[rows] = np.sqrt(np.clip(pooled, AWX_EPS, 1.0 - AWX_EPS))
    return (lo_full, awx_full), res


def kernel(x, W0, b0, W1, b1, W2, b2, R):
    out, _ = _run(x, W0, b0, W1, b1, W2, b2, R, trace=False)
    return out
